# revision 8
# baseline (speedup 1.0000x reference)
"""Trainium2 Bass kernel for ConstituencyMFVI.

Reference computation (per batch b):
    mask2o[i,j,k] = mask[i,j] & (min(i,j) != k) & (max(i,j) != k)
    A = s_pair * mask2o                       # [L, L, L]; per (b,i): A_i = [j, k]
    q = s_span
    repeat 3x:  q[i,j] = s_span[i,j] + sum_k A[i,j,k] * sigmoid(q)[i,k]
    out = sigmoid(q)

Strategy: pure data parallel over batch (4 batches per core on 8 cores).
Host marshals inputs (folds the deterministic mask2o, casts to fp16 —
11-bit mantissa keeps final abs err ~4e-3). The device computes each MFVI
iteration per batch with two fat DVE instructions in natural layout:

    T[i, j, k] = SP[i, j, k] * V[i, k]     (tensor_mul, V broadcast over j
                                            via a stride-0 AP dim)
    qred[i, j] = sum_k T[i, j, k]          (tensor_reduce axis=X)
    q = qred + s_span; V' = sigmoid(q)     (DVE add + ScalarE sigmoid)

This execution environment is dominated by fixed per-instruction overhead
(~80us/instruction measured, regardless of instruction size), so the kernel
minimizes instruction count (~56 total) with maximally large operations.
"""

import numpy as np

import concourse.bacc as bacc
import concourse.mybir as mybir
import concourse.tile as tile
from concourse.bass_utils import run_bass_kernel_spmd

N_CORES = 8
B, L = 32, 128
BPC = B // N_CORES  # batches per core
MAX_ITER = 3

_cached = {}


def build_nc(repeats=1):
    nc = bacc.Bacc("TRN2", target_bir_lowering=False, debug=False)
    sp = nc.dram_tensor("sp", [BPC, L, L, L], mybir.dt.float16, kind="ExternalInput")
    ss = nc.dram_tensor("ss", [BPC, L, L], mybir.dt.float32, kind="ExternalInput")
    out = nc.dram_tensor("out", [BPC, L, L], mybir.dt.float32, kind="ExternalOutput")

    with tile.TileContext(nc) as tc:
        with (
            tc.tile_pool(name="atp", bufs=1) as atp,
            tc.tile_pool(name="tp", bufs=1) as tp,
            tc.tile_pool(name="misc", bufs=2) as misc,
        ):
            for r in range(repeats):
                # SP: [i(part), b, j, k] fp16 — one DMA for all batches.
                spt = atp.tile(
                    [L, BPC, L, L], mybir.dt.float16, name=f"spt_{r}", tag="spt",
                )
                nc.sync.dma_start(spt[:], sp[:].transpose([1, 0, 2, 3]))
                # s_span: [i(part), b, j] f32 — one DMA.
                sst = misc.tile(
                    [L, BPC, L], mybir.dt.float32, name=f"sst_{r}", tag="sst"
                )
                nc.sync.dma_start(sst[:], ss[:].transpose([1, 0, 2]))

                # V0 = sigmoid(s_span), all batches in one ACT op: [i, b, k] fp16
                v = misc.tile([L, BPC, L], mybir.dt.float16, name=f"v0_{r}", tag="v")
                nc.scalar.activation(
                    v[:], sst[:], mybir.ActivationFunctionType.Sigmoid
                )

                for t in range(MAX_ITER):
                    last = t == MAX_ITER - 1
                    qred = misc.tile(
                        [L, BPC, L], mybir.dt.float32, name=f"qred_{t}_{r}", tag="qred"
                    )
                    for b in range(BPC):
                        tmp = tp.tile(
                            [L, L, L], mybir.dt.float16, name=f"tmp{b}_{t}_{r}",
                            tag="tmp",
                        )
                        nc.vector.tensor_mul(
                            tmp[:],
                            spt[:, b],
                            v[:, b].unsqueeze(1).broadcast_to([L, L, L]),
                        )
                        nc.vector.tensor_reduce(
                            qred[:, b],
                            tmp[:],
                            axis=mybir.AxisListType.X,
                            op=mybir.AluOpType.add,
                        )
                    q = misc.tile(
                        [L, BPC, L], mybir.dt.float32, name=f"q_{t}_{r}", tag="q"
                    )
                    nc.vector.tensor_add(q[:], qred[:], sst[:])
                    v = misc.tile(
                        [L, BPC, L],
                        mybir.dt.float32 if last else mybir.dt.float16,
                        name=f"v_{t}_{r}",
                        tag="vf" if last else "v",
                    )
                    nc.scalar.activation(
                        v[:], q[:], mybir.ActivationFunctionType.Sigmoid
                    )
                # out dram [b, i, j] <- v [i(part), b, j]
                nc.sync.dma_start(out[:].transpose([1, 0, 2]), v[:])
    nc.compile()
    return nc


def _prep(s_span, s_pair, mask):
    """Host-side marshalling: mask2o fold + fp16 cast (natural layout)."""
    s_span = np.ascontiguousarray(np.asarray(s_span, dtype=np.float32))
    mask = np.asarray(mask)
    sp = np.asarray(s_pair).astype(np.float16)
    if not mask.all():
        sp *= mask[:, :, :, None]
    idx = np.arange(L)
    ii, jj = np.meshgrid(idx, idx, indexing="ij")
    sp[:, ii, jj, np.minimum(ii, jj)] = 0.0
    sp[:, ii, jj, np.maximum(ii, jj)] = 0.0
    return sp, s_span


def kernel(s_span, s_pair, mask):
    if "nc" not in _cached:
        _cached["nc"] = build_nc()
    nc = _cached["nc"]

    sp16, ss32 = _prep(s_span, s_pair, mask)

    in_maps = []
    for c in range(N_CORES):
        lo, hi = c * BPC, (c + 1) * BPC
        in_maps.append(
            {
                "sp": np.ascontiguousarray(sp16[lo:hi]),
                "ss": np.ascontiguousarray(ss32[lo:hi]),
            }
        )

    res = run_bass_kernel_spmd(nc, in_maps, core_ids=list(range(N_CORES)))
    outs = [r["out"] for r in res.results]  # each [BPC, L, L]
    return np.ascontiguousarray(np.concatenate(outs, axis=0)).astype(np.float32)
